# revision 1
# baseline (speedup 1.0000x reference)
"""CapsRoute Trainium2 kernel v2: o-major routing, direct-to-channel votes.

Structure per row-chunk (nr rows, NC = nr*96 pixels):
  conv1 (18 mm) -> silu -> [o-major logits (4 mm) -> exp -> S (4 mm) ->
  recip -> th -> rtp (4 mm) -> ar -> asum (2 mm) -> as_eps -> r ->
  bc (2 mm) -> coeff] -> per-o [rep mm -> cp mul -> vote mm into
  channel-ordered packed psum] + act-insert mm -> Act copy -> 2 scatter
  DMAs per half into y2 -> conv2 (18 mm) at lag 3.
"""
import numpy as np
import concourse.bass as bass
import concourse.bacc as bacc
import concourse.tile as tile
from concourse import mybir
from concourse._compat import with_exitstack
from contextlib import ExitStack

K_CAT, P_CAT, K_OUT, P_OUT = 16, 8, 16, 8
C_CAT = 144
EPS_ROUTE = 1e-6
BN_EPS = 1e-5
H = W = 96
HP = WP = 98
ROWS_PER_CHUNK = 5
CHUNKS = [(5 * i, 5) for i in range(16)] + [(80 + 4 * i, 4) for i in range(4)]
NCHUNK = len(CHUNKS)
N = ROWS_PER_CHUNK * W  # 480
PSN = 512
C2LAG = 3

F32 = mybir.dt.float32
BF16 = mybir.dt.bfloat16
AF = mybir.ActivationFunctionType
ALU = mybir.AluOpType


def prep_constants(conv_route_w, conv_route_gamma, conv_route_beta,
                   W_pose, W_gate, b_gate, spagg_w, spagg_gamma, spagg_beta):
    out = {}
    scale1 = (conv_route_gamma / np.sqrt(1.0 + BN_EPS)).astype(np.float32)
    scale2 = (spagg_gamma / np.sqrt(1.0 + BN_EPS)).astype(np.float32)

    # conv1 lhsT [72, 2, 9, 72]: [cin_local, half, tap, col j]
    # col j: j<64 -> pose (k_loc=j//8, p=j%8); j>=64 -> act of k_loc=j-64
    c1 = np.zeros((72, 2, 9, 72), np.float32)
    w1 = conv_route_w * scale1[:, None, None, None]
    b1 = np.zeros((2, 72), np.float32)
    for h in range(2):
        for j in range(72):
            k_loc, ch = (j // 8, j % 8) if j < 64 else (j - 64, 8)
            cout = 72 * h + 9 * k_loc + ch
            for ci in range(9):
                for dy in range(3):
                    for dx in range(3):
                        c1[9 * k_loc + ci, h, 3 * dy + dx, j] = w1[cout, ci, dy, dx]
            b1[h, j] = conv_route_beta[cout]
    out["c1w"] = c1
    out["b1"] = b1

    # o-major logits lhsT [64, 4, 128]: slots (half-tile, rhs-half):
    # 0=(A,pose0) 1=(A,ptmp) 2=(B,pose0) 3=(B,ptmp); row r=8*k_loc+p,
    # col j=16*o_loc+k
    wg4 = np.zeros((64, 4, 128), np.float32)
    for r in range(64):
        kl, p = r // 8, r % 8
        for o_loc in range(8):
            wg4[r, 0, 16 * o_loc + kl] = W_gate[kl, o_loc, p]
            wg4[r, 1, 16 * o_loc + 8 + kl] = W_gate[8 + kl, o_loc, p]
            wg4[r, 2, 16 * o_loc + kl] = W_gate[kl, 8 + o_loc, p]
            wg4[r, 3, 16 * o_loc + 8 + kl] = W_gate[8 + kl, 8 + o_loc, p]
    out["wg4"] = wg4

    bg = np.asarray(b_gate).reshape(K_CAT, K_OUT)
    bgA = np.zeros((128, 1), np.float32)
    bgB = np.zeros((128, 1), np.float32)
    for o_loc in range(8):
        for k in range(16):
            bgA[16 * o_loc + k, 0] = bg[k, o_loc]
            bgB[16 * o_loc + k, 0] = bg[k, 8 + o_loc]
    out["bgA"] = bgA
    out["bgB"] = bgB

    # S: sum E over o per k-half: rows 16o+k -> col k-8h
    onesS = np.zeros((128, 2, 8), np.float32)
    for o_loc in range(8):
        for k in range(16):
            h = k // 8
            onesS[16 * o_loc + k, h, k - 8 * h] = 1.0
    out["onesS"] = onesS

    # rtp broadcast: rows 64+kl (th rows) -> cols 16o+ (8h+kl); 0.5 folds
    # the tanh->sigmoid affine
    rt = np.zeros((72, 2, 128), np.float32)
    for h in range(2):
        for kl in range(8):
            for o_loc in range(8):
                rt[64 + kl, h, 16 * o_loc + 8 * h + kl] = 0.5
    out["rt"] = rt

    # asum: rows 16o+k -> col o
    onesA = np.zeros((128, 8), np.float32)
    for o_loc in range(8):
        for k in range(16):
            onesA[16 * o_loc + k, o_loc] = 1.0
    out["onesA"] = onesA

    # late-normalize broadcast: r rows (A 0:8 / B 32:40, row 8 = ones) ->
    # packed channel cols; act cols get the constant-1 row
    rsel2 = np.zeros((72, 2, 100), np.float32)
    for h in range(2):
        for o_loc in range(8):
            for q in range(8):
                col = 64 * (o_loc // 4) + 9 * (o_loc % 4) + q
                rsel2[32 * h + o_loc, h, col] = 1.0
        for g in range(2):
            for j in range(4):
                rsel2[64, h, 64 * g + 9 * j + 8] = 1.0
    out["rsel2"] = rsel2

    # rep broadcast: rows 16o'+k -> cols 8k+p (all p) for o'==o_loc
    sel2 = np.zeros((128, 8, 128), np.float32)
    for o_loc in range(8):
        for k in range(16):
            for p in range(8):
                sel2[16 * o_loc + k, o_loc, 8 * k + p] = 1.0
    out["sel2"] = sel2

    # votes straight into channel order: rows 8k+p, per o: col
    # 9*(o_loc%4)+q in group o_loc//4
    wp36 = np.zeros((128, 16, 36), np.float32)
    for o in range(16):
        o_loc = o % 8
        for k in range(16):
            for p in range(8):
                for q in range(8):
                    wp36[8 * k + p, o, 9 * (o_loc % 4) + q] = W_pose[k, o, p, q]
    out["wp36"] = wp36

    # act insertion: rows of as_eps (A 0:8 / B 32:40) -> cols {9j+8, 64+9j+8}
    actsel = np.zeros((40, 2, 100), np.float32)
    for h in range(2):
        for o_loc in range(8):
            col = 64 * (o_loc // 4) + 9 * (o_loc % 4) + 8
            actsel[32 * h + o_loc, h, col] = 1.0
    out["actsel"] = actsel

    # conv2 lhsT [72, 2, 9, 72] natural channel order
    c2 = np.zeros((72, 2, 9, 72), np.float32)
    w2 = spagg_w * scale2[:, None, None, None]
    for h in range(2):
        for j in range(72):
            cout = 72 * h + j
            g_loc = j // 9
            for ci in range(9):
                for dy in range(3):
                    for dx in range(3):
                        c2[9 * g_loc + ci, h, 3 * dy + dx, j] = w2[cout, ci, dy, dx]
    out["c2w"] = c2
    out["b2"] = spagg_beta.reshape(2, 72).astype(np.float32)
    for name, (shape, dt) in CONST_SPECS.items():
        want = mybir.dt.np(dt)
        out[name] = np.ascontiguousarray(out[name]).astype(want)
    return out


CONST_SPECS = {
    "c1w": ([72, 2, 9, 72], BF16),
    "b1": ([2, 72], F32),
    "wg4": ([64, 4, 128], BF16),
    "bgA": ([128, 1], F32),
    "bgB": ([128, 1], F32),
    "onesS": ([128, 2, 8], BF16),
    "rt": ([72, 2, 128], BF16),
    "onesA": ([128, 8], BF16),
    "rsel2": ([72, 2, 100], BF16),
    "sel2": ([128, 8, 128], BF16),
    "wp36": ([128, 16, 36], BF16),
    "actsel": ([40, 2, 100], BF16),
    "c2w": ([72, 2, 9, 72], BF16),
    "b2": ([2, 72], F32),
}
BF16_NP = mybir.dt.np(BF16)
XROWS = [(0, 6), (6, 5), (11, 24), (35, 24), (59, 24), (83, 13)]


@with_exitstack
def capsroute_kernel(ctx: ExitStack, tc: tile.TileContext, outs, ins):
    nc = tc.nc
    out = outs["out"]

    singles = ctx.enter_context(tc.tile_pool(name="singles", bufs=1))
    xpool = ctx.enter_context(tc.tile_pool(name="xpool", bufs=1))
    y2pool = ctx.enter_context(tc.tile_pool(name="y2pool", bufs=1))
    work = ctx.enter_context(tc.tile_pool(name="work", bufs=5))
    psc = ctx.enter_context(tc.tile_pool(name="psc", bufs=2, space="PSUM"))
    psl = ctx.enter_context(tc.tile_pool(name="psl", bufs=2, space="PSUM"))
    psr = ctx.enter_context(tc.tile_pool(name="psr", bufs=2, space="PSUM"))
    pck = ctx.enter_context(tc.tile_pool(name="pck", bufs=2, space="PSUM"))

    cst = {}
    for name, (shape, dt) in CONST_SPECS.items():
        if name in ("b1", "b2"):
            continue
        t = singles.tile(shape, dt, name=f"{name}_c")
        cst[name] = t
    b1_t = [singles.tile([72, 1], F32, name=f"b1_{h}") for h in range(2)]
    b2_t = [singles.tile([72, 1], F32, name=f"b2_{h}") for h in range(2)]

    zmm = singles.tile([1, 128], BF16, name="zmm")
    nc.vector.memset(zmm[:], 0.0)
    zrhs = singles.tile([1, N], BF16, name="zrhs")
    nc.vector.memset(zrhs[:], 0.0)
    r_t = singles.tile([72, N], BF16, name="r_t")
    nc.vector.memset(r_t[:], 0.0)
    nc.vector.memset(r_t[64:72, :], 1.0)
    xpad = [xpool.tile([72, HP, WP], BF16, name=f"xpad{h}") for h in range(2)]
    y2 = [y2pool.tile([72, HP, WP], BF16, name=f"y2{h}") for h in range(2)]

    def pad_border(t):
        nc.vector.memset(t[:, 0, :], 0.0)
        nc.vector.memset(t[:, 97, :], 0.0)
        nc.vector.memset(t[:, :, 0:1], 0.0)
        nc.vector.memset(t[:, :, 97:98], 0.0)

    for h in range(2):
        pad_border(xpad[h])
        pad_border(y2[h])

    nc.sync.dma_start(out=cst["c1w"][:, 0, 0:3], in_=ins["c1w"][:, 0, 0:3])
    nc.sync.dma_start(out=cst["c1w"][:, 0, 3:9], in_=ins["c1w"][:, 0, 3:9])
    r0, nr = XROWS[0]
    nc.gpsimd.dma_start(out=xpad[0][:, 1 + r0:1 + r0 + nr, 1:97],
                        in_=ins["x0"][:, r0:r0 + nr, :])
    nc.sync.dma_start(out=cst["c1w"][:, 1], in_=ins["c1w"][:, 1])
    nc.gpsimd.dma_start(out=xpad[1][:, 1 + r0:1 + r0 + nr, 1:97],
                        in_=ins["x1"][:, r0:r0 + nr, :])
    for h in range(2):
        nc.sync.dma_start(out=b1_t[h][:], in_=ins["b1"][h:h + 1, :].transpose([1, 0]))
    for name in ("wg4", "bgA", "bgB", "onesS", "rt", "onesA", "rsel2"):
        nc.sync.dma_start(out=cst[name][:], in_=ins[name][:])
    for h, xsrc in enumerate((ins["x0"], ins["x1"])):
        for r0, nr in XROWS[1:]:
            nc.gpsimd.dma_start(out=xpad[h][:, 1 + r0:1 + r0 + nr, 1:97],
                                in_=xsrc[:, r0:r0 + nr, :])
    for name in ("sel2", "wp36", "actsel"):
        nc.sync.dma_start(out=cst[name][:], in_=ins[name][:])
    nc.sync.dma_start(out=cst["c2w"][:], in_=ins["c2w"][:])
    for h in range(2):
        nc.sync.dma_start(out=b2_t[h][:], in_=ins["b2"][h:h + 1, :].transpose([1, 0]))

    def win(t, r0, nr, dy, dx):
        rs = 1 + r0 + dy
        return t[:, rs:rs + nr, 1 + dx:1 + dx + W]

    def conv_thunks(c, src_tiles, wname, tag):
        """Create psum tiles + 18 one-tap matmul thunks (emitted lazily)."""
        r0, nr = CHUNKS[c]
        NC = nr * W
        ps_h = [psc.tile([72, PSN], F32, tag="conv", name=tag) for _ in range(2)]
        thunks = []
        for h in range(2):
            for tap in range(9):
                def t(h=h, tap=tap, ps=ps_h[h], r0=r0, nr=nr, NC=NC):
                    dy, dx = tap // 3 - 1, tap % 3 - 1
                    nc.tensor.matmul(
                        ps[:, 0:NC], cst[wname][:, h, tap],
                        win(src_tiles[h], r0, nr, dy, dx),
                        start=(tap == 0), stop=(tap == 8))
                thunks.append(t)
        return ps_h, thunks

    def silu_block(c, c1ps, c2fin):
        """Act silu-table block: conv2(c-3) silus + conv1(c) silus; then
        tanh (exp-table). Emits conv2 out DMA + pose-copy DMA."""
        r0, nr = CHUNKS[c]
        NC = nr * W
        if c2fin is not None:
            c2c, c2ps = c2fin
            r2, nr2 = CHUNKS[c2c]
            NC2 = nr2 * W
            for h in range(2):
                ob = work.tile([72, N], F32, tag="ob")
                nc.scalar.activation(ob[:, 0:NC2], c2ps[h][:, 0:NC2], AF.Silu,
                                     bias=b2_t[h][:])
                nc.sync.dma_start(
                    out=out[72 * h:72 * h + 72, r2:r2 + nr2, :],
                    in_=ob[:, 0:NC2].rearrange("p (r w) -> p r w", w=W))
        pose = work.tile([128, N], BF16, tag="pose", bufs=4)
        ptmp = work.tile([64, N], BF16, tag="ptmp")
        acty = [work.tile([72, N], F32, name=f"acty{h}", tag=f"acty{h}")
                for h in range(2)]
        nc.scalar.activation(pose[0:64, 0:NC], c1ps[0][0:64, 0:NC], AF.Silu,
                             bias=b1_t[0][0:64, :])
        nc.scalar.activation(ptmp[:, 0:NC], c1ps[1][0:64, 0:NC], AF.Silu,
                             bias=b1_t[1][0:64, :])
        for h in range(2):
            nc.scalar.activation(acty[h][64:72, 0:NC], c1ps[h][64:72, 0:NC],
                                 AF.Silu, bias=b1_t[h][64:72, :])
        nc.sync.dma_start(out=pose[64:128, 0:NC], in_=ptmp[:, 0:NC])
        tt = [work.tile([72, N], BF16, name=f"t{h}", tag=f"t{h}") for h in range(2)]
        for h in range(2):
            with nc.allow_low_precision(reason="bf16 routing"):
                nc.scalar.activation(tt[h][64:72, 0:NC], acty[h][64:72, 0:NC],
                                     AF.Tanh, scale=0.5)
        return pose, ptmp, tt

    def routing_body(c, sil, take):
        """Routing for chunk c; `take(n)` emits n filler matmuls (conv2(c-3)
        then conv1(c+1)) to pad PE stalls on the Act/DVE chain."""
        r0, nr = CHUNKS[c]
        NC = nr * W
        pose, ptmp, tt = sil
        E = []
        for s, half in enumerate("AB"):
            L = psl.tile([128, PSN], F32, tag="L", name=f"L{half}")
            nc.tensor.matmul(L[:, 0:NC], cst["wg4"][:, 2 * s], pose[0:64, 0:NC],
                             start=True, stop=False)
            nc.tensor.matmul(L[:, 0:NC], cst["wg4"][:, 2 * s + 1], ptmp[:, 0:NC],
                             start=False, stop=True)
            Eh = work.tile([128, N], BF16, tag=f"E{half}", bufs=4)
            with nc.allow_low_precision(reason="bf16 routing"):
                nc.scalar.activation(Eh[:, 0:NC], L[:, 0:NC], AF.Exp,
                                     bias=cst["bg" + half][:])
            E.append(Eh)
        take(5)
        Sth = [psl.tile([72, PSN], F32, tag="L", name=f"S{h}") for h in range(2)]
        for h in range(2):
            nc.tensor.matmul(Sth[h][64:72, 0:NC], cst["onesS"][:, h], E[0][:, 0:NC],
                             start=True, stop=False)
            nc.tensor.matmul(Sth[h][64:72, 0:NC], cst["onesS"][:, h], E[1][:, 0:NC],
                             start=False, stop=True)
        th = []
        for h in range(2):
            rS = work.tile([72, N], F32, name=f"rS{h}", tag=f"rS{h}")
            nc.vector.reciprocal(rS[64:72, 0:NC], Sth[h][64:72, 0:NC])
            t2 = work.tile([72, N], BF16, name=f"th{h}", tag=f"th{h}")
            with nc.allow_low_precision(reason="bf16 routing"):
                nc.vector.scalar_tensor_tensor(t2[64:72, 0:NC], tt[h][64:72, 0:NC],
                                               1.0, rS[64:72, 0:NC],
                                               op0=ALU.add, op1=ALU.mult)
            th.append(t2)
        take(5)
        ar = []
        for s, half in enumerate("AB"):
            rtp = psl.tile([128, PSN], F32, tag="L", name=f"rtp{half}")
            nc.tensor.matmul(rtp[:, 0:NC], cst["rt"][64:72, 0], th[0][64:72, 0:NC],
                             start=True, stop=False)
            nc.tensor.matmul(rtp[:, 0:NC], cst["rt"][64:72, 1], th[1][64:72, 0:NC],
                             start=False, stop=True)
            arh = work.tile([128, N], BF16, name=f"ar{half}", tag=f"ar{half}", bufs=4)
            with nc.allow_low_precision(reason="bf16 routing"):
                nc.vector.tensor_mul(arh[:, 0:NC], E[s][:, 0:NC], rtp[:, 0:NC])
            ar.append(arh)
        take(4)
        asumA = psl.tile([8, PSN], F32, tag="L", name="asumA")
        asumB = psl.tile([40, PSN], F32, tag="L", name="asumB")
        nc.tensor.matmul(asumA[0:8, 0:NC], cst["onesA"][:], ar[0][:, 0:NC],
                         start=True, stop=True)
        nc.tensor.matmul(asumB[32:40, 0:NC], cst["onesA"][:], ar[1][:, 0:NC],
                         start=True, stop=True)
        as_eps = work.tile([40, N], BF16, tag="as_eps")
        with nc.allow_low_precision(reason="bf16 act channel"):
            nc.scalar.activation(as_eps[0:8, 0:NC], asumA[0:8, 0:NC], AF.Copy,
                                 bias=EPS_ROUTE)
            nc.scalar.activation(as_eps[32:40, 0:NC], asumB[32:40, 0:NC], AF.Copy,
                                 bias=EPS_ROUTE)
        r = r_t
        with nc.allow_low_precision(reason="bf16 routing"):
            nc.vector.reciprocal(r[0:8, 0:NC], as_eps[0:8, 0:NC])
            nc.vector.reciprocal(r[32:40, 0:NC], as_eps[32:40, 0:NC])
        take(4)
        rrep_sb = []
        for s in range(2):
            rrep = pck.tile([128, PSN], F32, tag="pck", name=f"rrep{s}")
            nc.tensor.matmul(rrep[0:100, 0:NC], cst["rsel2"][:, s], r[0:72, 0:NC],
                             start=True, stop=True)
            rsb = work.tile([128, N], BF16, name=f"rrsb{s}", tag=f"rrsb{s}")
            with nc.allow_low_precision(reason="bf16 routing"):
                nc.scalar.copy(rsb[0:100, 0:NC], rrep[0:100, 0:NC])
            rrep_sb.append(rsb)
        pk_sb = []
        for h in range(2):
            packed = pck.tile([128, PSN], F32, tag="pck", name=f"pk{h}")
            nc.tensor.matmul(packed[0:100, 0:NC], zmm[0:1, 0:100], zrhs[0:1, 0:NC],
                             start=True, stop=False, skip_group_check=True)
            for o_loc in range(8):
                o = 8 * h + o_loc
                g = o_loc // 4
                rep = psr.tile([128, PSN], F32, tag="rep", name="rep")
                nc.tensor.matmul(rep[:, 0:NC], cst["sel2"][:, o_loc],
                                 ar[h][:, 0:NC], start=True, stop=True)
                cp = work.tile([128, N], BF16, tag="cp", bufs=6)
                with nc.allow_low_precision(reason="bf16 routing"):
                    nc.vector.tensor_mul(cp[:, 0:NC], pose[:, 0:NC], rep[:, 0:NC])
                nc.tensor.matmul(packed[64 * g:64 * g + 36, 0:NC], cst["wp36"][:, o],
                                 cp[:, 0:NC], start=False, stop=False,
                                 skip_group_check=True, tile_position=(0, 64 * g))
                take(1)
            nc.tensor.matmul(packed[0:100, 0:NC], cst["actsel"][32 * h:32 * h + 8, h],
                             as_eps[32 * h:32 * h + 8, 0:NC], start=False, stop=True,
                             skip_group_check=True, tile_position=(32 * h, 0))
            pk_sb.append((packed, rrep_sb[h]))
        take(100)
        return pk_sb

    def pk_copies(c, pk_sb):
        r0, nr = CHUNKS[c]
        NC = nr * W
        outp = []
        for h, (packed, rsb) in enumerate(pk_sb):
            pk = work.tile([128, N], BF16, tag=f"pk{h}", bufs=3)
            with nc.allow_low_precision(reason="bf16 conv2 input"):
                nc.vector.tensor_mul(pk[0:100, 0:NC], rsb[0:100, 0:NC],
                                     packed[0:100, 0:NC])
            outp.append(pk)
        return outp

    def scatter_chunk(c, pks):
        r0, nr = CHUNKS[c]
        NC = nr * W
        for h in range(2):
            for g in range(2):
                src = pks[h][64 * g:64 * g + 36, 0:NC].rearrange(
                    "p (r w) -> p r w", w=W)
                nc.sync.dma_start(
                    out=y2[h][36 * g:36 * g + 36, 1 + r0:1 + r0 + nr, 1:97],
                    in_=src)

    # ---- software-pipelined main loop ----
    c1ps, c1thunks = conv_thunks(0, xpad, "c1w", "c1ps")
    for t in c1thunks:
        t()
    sil = silu_block(0, c1ps, None)
    for c in range(NCHUNK):
        filler = []
        c2state = None
        if c >= C2LAG:
            c2ps, c2thunks = conv_thunks(c - C2LAG, y2, "c2w", "c2ps")
            filler += c2thunks
            c2state = (c - C2LAG, c2ps)
        nxt = None
        if c + 1 < NCHUNK:
            nxt_ps, nxt_thunks = conv_thunks(c + 1, xpad, "c1w", "c1ps")
            filler += nxt_thunks
            nxt = nxt_ps
        extra_fin = None
        if c == NCHUNK - 1:
            e_ps, e_thunks = conv_thunks(c - C2LAG + 1, y2, "c2w", "c2ps")
            filler += e_thunks
            extra_fin = (c - C2LAG + 1, e_ps)
        it = iter(filler)

        def take(n):
            for _ in range(n):
                t = next(it, None)
                if t is None:
                    return
                t()

        pk_sb = routing_body(c, sil, take)
        if c + 1 < NCHUNK:
            sil = silu_block(c + 1, nxt, c2state)
        elif c2state is not None:
            c2c, c2ps = c2state
            r2, nr2 = CHUNKS[c2c]
            NC2 = nr2 * W
            for h in range(2):
                ob = work.tile([72, N], F32, tag="ob")
                nc.scalar.activation(ob[:, 0:NC2], c2ps[h][:, 0:NC2], AF.Silu,
                                     bias=b2_t[h][:])
                nc.sync.dma_start(
                    out=out[72 * h:72 * h + 72, r2:r2 + nr2, :],
                    in_=ob[:, 0:NC2].rearrange("p (r w) -> p r w", w=W))
        pks = pk_copies(c, pk_sb)
        scatter_chunk(c, pks)
        if c == NCHUNK - 1 and extra_fin is not None:
            c2c, c2ps = extra_fin
            r2, nr2 = CHUNKS[c2c]
            NC2 = nr2 * W
            for h in range(2):
                ob = work.tile([72, N], F32, tag="ob")
                nc.scalar.activation(ob[:, 0:NC2], c2ps[h][:, 0:NC2], AF.Silu,
                                     bias=b2_t[h][:])
                nc.sync.dma_start(
                    out=out[72 * h:72 * h + 72, r2:r2 + nr2, :],
                    in_=ob[:, 0:NC2].rearrange("p (r w) -> p r w", w=W))

    # drain remaining conv2 chunks
    for c2c in range(NCHUNK - C2LAG + 1, NCHUNK):
        c2ps, c2thunks = conv_thunks(c2c, y2, "c2w", "c2ps")
        for t in c2thunks:
            t()
        r2, nr2 = CHUNKS[c2c]
        NC2 = nr2 * W
        for h in range(2):
            ob = work.tile([72, N], F32, tag="ob")
            nc.scalar.activation(ob[:, 0:NC2], c2ps[h][:, 0:NC2], AF.Silu,
                                 bias=b2_t[h][:])
            nc.sync.dma_start(
                out=out[72 * h:72 * h + 72, r2:r2 + nr2, :],
                in_=ob[:, 0:NC2].rearrange("p (r w) -> p r w", w=W))


def build_nc():
    nc = bacc.Bacc("TRN2", target_bir_lowering=False, debug=False)
    ins = {
        "x0": nc.dram_tensor("x0", [72, H, W], BF16, kind="ExternalInput").ap(),
        "x1": nc.dram_tensor("x1", [72, H, W], BF16, kind="ExternalInput").ap(),
    }
    for name, (shape, dt) in CONST_SPECS.items():
        ins[name] = nc.dram_tensor(name, shape, dt, kind="ExternalInput").ap()
    outs = {"out": nc.dram_tensor("out", [C_CAT, H, W], F32, kind="ExternalOutput").ap()}
    with tile.TileContext(nc) as tc:
        capsroute_kernel(tc, outs, ins)
    nc.compile()
    return nc


_NC_CACHE = {}


def _get_nc():
    if "nc" not in _NC_CACHE:
        _NC_CACHE["nc"] = build_nc()
    return _NC_CACHE["nc"]


def kernel(**inputs):
    """Full-batch entry point: shards batch 8 across 8 NeuronCores."""
    from concourse import bass_utils

    nc = _get_nc()
    consts = prep_constants(
        inputs["conv_route_w"].astype(np.float32),
        inputs["conv_route_gamma"].astype(np.float32),
        inputs["conv_route_beta"].astype(np.float32),
        inputs["W_pose"].astype(np.float32),
        inputs["W_gate"].astype(np.float32),
        inputs["b_gate"].astype(np.float32),
        inputs["spagg_w"].astype(np.float32),
        inputs["spagg_gamma"].astype(np.float32),
        inputs["spagg_beta"].astype(np.float32),
    )
    x0 = np.asarray(inputs["x0"]).astype(BF16_NP)
    x1 = np.asarray(inputs["x1"]).astype(BF16_NP)
    in_maps = []
    for b in range(8):
        m = dict(consts)
        m["x0"] = np.ascontiguousarray(x0[b])
        m["x1"] = np.ascontiguousarray(x1[b])
        in_maps.append(m)
    res = bass_utils.run_bass_kernel_spmd(nc, in_maps, core_ids=list(range(8)))
    out = np.stack([res.results[b]["out"] for b in range(8)], axis=0)
    return out.astype(np.float32)

